# revision 7
# baseline (speedup 1.0000x reference)
"""Trainium2 Bass kernel for nn_DecoupledRouterPool (dense-MoE router).

Math (per expert c, per token b):
  h1 = gelu(LN(W1 x; g1, be1))       [512 -> 256]
  h2 = gelu(LN(W2 h1; g2, be2))      [256 -> 256]
  t  = W3 h2 + b3                    [256 -> 512]
  sim  = <t, a_c> / (|t| |a_c|)
  dist = |t/|t| - a_c/|a_c|| = sqrt(2 - 2 sim)
  routed = argmin_c dist

Sharding: experts split across 8 cores (100 -> pad 104 -> 13/core), x replicated.

Host-side folding (exact algebra, done in fp32 numpy):
  - LN mean subtraction folded into centered weights: W~ = W - colmean(W)
  - LN gain g folded into W~ rows; variance recovered with weighted reduce
    1/(H g^2) on the squared pre-activations.
  - anchors normalized on host; dot(t, a_hat) folded into a reduction vector
    w_dot = W3^T a_hat (plus constant a_hat . b3).

Device layout is feature-major ([feature, token]) throughout => no transposes.
Per-token statistics are reduced on the TensorEngine with column-placed lhsT
vectors so each expert's stat row lands on its own PSUM partition, giving
batched [13, 512] rows for the ACT rsqrt (Exp(-0.5 Ln(v+eps))) computation.
"""

import numpy as np

B = 2048
IN = 512
HID = 256
OUT = 512
C = 100
NCORES = 8
EL = 13          # experts per core (104 padded)
CPAD = EL * NCORES
P = 128
CHUNK = 512
NB = B // CHUNK
EPS = 1e-5

_PROG = None  # cached (nc, meta)


# --------------------------------------------------------------------------
# host-side preprocessing
# --------------------------------------------------------------------------

def _prep(inputs):
    f = np.float32
    x = np.asarray(inputs["x"], f)
    W1 = np.asarray(inputs["W1"], f); b1 = np.asarray(inputs["b1"], f)
    g1 = np.asarray(inputs["g1"], f); be1 = np.asarray(inputs["be1"], f)
    W2 = np.asarray(inputs["W2"], f); b2 = np.asarray(inputs["b2"], f)
    g2 = np.asarray(inputs["g2"], f); be2 = np.asarray(inputs["be2"], f)
    W3 = np.asarray(inputs["W3"], f); b3 = np.asarray(inputs["b3"], f)
    anchors = np.asarray(inputs["anchors"], f)

    pad = np.arange(CPAD); pad[pad >= C] = C - 1  # replicate last expert

    W1 = W1[pad]; b1 = b1[pad]; g1 = g1[pad]; be1 = be1[pad]
    W2 = W2[pad]; b2 = b2[pad]; g2 = g2[pad]; be2 = be2[pad]
    W3 = W3[pad]; b3 = b3[pad]; anchors = anchors[pad]

    # centered + gain-folded layer weights
    W1c = (W1 - W1.mean(axis=1, keepdims=True)) * g1[:, :, None]  # [C,H,IN]
    b1c = (b1 - b1.mean(axis=1, keepdims=True)) * g1              # [C,H]
    W2c = (W2 - W2.mean(axis=1, keepdims=True)) * g2[:, :, None]
    b2c = (b2 - b2.mean(axis=1, keepdims=True)) * g2

    # variance reduce weights: var = sum_f p_f^2 / (H g_f^2)
    wv1 = (1.0 / (HID * np.maximum(g1 * g1, 1e-30))).astype(f)    # [C,H]
    wv2 = (1.0 / (HID * np.maximum(g2 * g2, 1e-30))).astype(f)

    an = anchors / np.linalg.norm(anchors, axis=-1, keepdims=True)  # [C,OUT]
    wd = np.einsum("co,coh->ch", an, W3).astype(f)                  # [C,H]
    cdot = np.einsum("co,co->c", an, b3).astype(f)                  # [C]

    xt = np.ascontiguousarray(x.T.reshape(4, P, B))                 # [4,P,B]

    per_core = []
    for k in range(NCORES):
        sl = slice(k * EL, (k + 1) * EL)
        w1t = np.ascontiguousarray(
            W1c[sl].transpose(0, 2, 1).reshape(EL, 4, P, HID))      # [EL,4,P,H]
        w2t = np.ascontiguousarray(
            W2c[sl].transpose(0, 2, 1).reshape(EL, 2, P, HID))
        w3t = np.ascontiguousarray(
            W3[sl].transpose(0, 2, 1).reshape(EL, 2, P, OUT))

        def wide(vec, kt):  # vec [EL, kt*P] -> [EL, kt, P, 32] col c placed
            out = np.zeros((EL, kt, P, 32), f)
            v = vec.reshape(EL, kt, P)
            for c in range(EL):
                out[c, :, :, c] = v[c]
            return out

        wv1w = wide(wv1[sl], 2)
        wv2w = wide(wv2[sl], 2)
        wdw = wide(wd[sl], 2)
        sww = wide(np.ones((EL, OUT), f), 4)

        ecol = np.zeros((EL, 32, P), f)
        for c in range(EL):
            ecol[c, c, :] = 1.0

        b1cc = np.ascontiguousarray(b1c[sl].reshape(EL, 2, P).transpose(0, 2, 1))  # [EL,P,2]
        b2cc = np.ascontiguousarray(b2c[sl].reshape(EL, 2, P).transpose(0, 2, 1))
        b3cc = np.ascontiguousarray(b3[sl].reshape(EL, 4, P).transpose(0, 2, 1))   # [EL,P,4]
        be1c = np.ascontiguousarray(be1[sl].reshape(EL, 2, P).transpose(0, 2, 1))
        be2c = np.ascontiguousarray(be2[sl].reshape(EL, 2, P).transpose(0, 2, 1))
        cd = np.zeros((32, 1), f); cd[:EL, 0] = cdot[sl]

        per_core.append({
            "xt": xt, "w1t": w1t, "w2t": w2t, "w3t": w3t,
            "wv1w": wv1w, "wv2w": wv2w, "wdw": wdw, "sww": sww,
            "ecol": ecol, "b1c": b1cc, "b2c": b2cc, "b3c": b3cc,
            "be1": be1c, "be2": be2c, "cdot": cd,
        })
    has_be = bool(np.any(be1) or np.any(be2))
    return per_core, has_be


# --------------------------------------------------------------------------
# device program
# --------------------------------------------------------------------------

def _build(has_be):
    from contextlib import ExitStack
    import concourse.bass as bass
    import concourse.bacc as bacc
    import concourse.tile as tile
    import concourse.mybir as mybir

    f32 = mybir.dt.float32
    AF = mybir.ActivationFunctionType
    ds = bass.ds

    nc = bacc.Bacc(target_bir_lowering=False)
    xt = nc.dram_tensor("xt", [4, P, B], f32, kind="ExternalInput")
    w1t = nc.dram_tensor("w1t", [EL, 4, P, HID], f32, kind="ExternalInput")
    w2t = nc.dram_tensor("w2t", [EL, 2, P, HID], f32, kind="ExternalInput")
    w3t = nc.dram_tensor("w3t", [EL, 2, P, OUT], f32, kind="ExternalInput")
    wv1w = nc.dram_tensor("wv1w", [EL, 2, P, 32], f32, kind="ExternalInput")
    wv2w = nc.dram_tensor("wv2w", [EL, 2, P, 32], f32, kind="ExternalInput")
    wdw = nc.dram_tensor("wdw", [EL, 2, P, 32], f32, kind="ExternalInput")
    sww = nc.dram_tensor("sww", [EL, 4, P, 32], f32, kind="ExternalInput")
    ecol = nc.dram_tensor("ecol", [EL, 32, P], f32, kind="ExternalInput")
    b1c = nc.dram_tensor("b1c", [EL, P, 2], f32, kind="ExternalInput")
    b2c = nc.dram_tensor("b2c", [EL, P, 2], f32, kind="ExternalInput")
    b3c = nc.dram_tensor("b3c", [EL, P, 4], f32, kind="ExternalInput")
    be1 = nc.dram_tensor("be1", [EL, P, 2], f32, kind="ExternalInput")
    be2 = nc.dram_tensor("be2", [EL, P, 2], f32, kind="ExternalInput")
    cdot = nc.dram_tensor("cdot", [32, 1], f32, kind="ExternalInput")
    sim_o = nc.dram_tensor("sim", [EL, B], f32, kind="ExternalOutput")
    dist_o = nc.dram_tensor("dist", [EL, B], f32, kind="ExternalOutput")

    with tile.TileContext(nc) as tc, ExitStack() as ctx:
        const = ctx.enter_context(tc.tile_pool(name="const", bufs=1))
        wpool = ctx.enter_context(tc.tile_pool(name="w", bufs=2))
        apool = ctx.enter_context(tc.tile_pool(name="a", bufs=14))
        sqpool = ctx.enter_context(tc.tile_pool(name="sq", bufs=3))
        stat = ctx.enter_context(tc.tile_pool(name="stat", bufs=4))
        outp = ctx.enter_context(tc.tile_pool(name="outp", bufs=1))
        fin = ctx.enter_context(tc.tile_pool(name="fin", bufs=4))
        pp = ctx.enter_context(tc.tile_pool(name="pp", bufs=2, space="PSUM"))
        pt = ctx.enter_context(tc.tile_pool(name="pt", bufs=2, space="PSUM"))
        pstat = ctx.enter_context(tc.tile_pool(name="ps", bufs=2, space="PSUM"))

        # ---- resident constants
        wv1s = const.tile([P, EL, 2, 32], f32)
        nc.sync.dma_start(wv1s, wv1w.rearrange("c k p m -> p c k m"))
        wv2s = const.tile([P, EL, 2, 32], f32)
        nc.sync.dma_start(wv2s, wv2w.rearrange("c k p m -> p c k m"))
        wds = const.tile([P, EL, 2, 32], f32)
        nc.sync.dma_start(wds, wdw.rearrange("c k p m -> p c k m"))
        sws = const.tile([P, EL, 4, 32], f32)
        nc.sync.dma_start(sws, sww.rearrange("c k p m -> p c k m"))
        ecols = const.tile([32, EL, P], f32)
        nc.sync.dma_start(ecols, ecol.rearrange("c e p -> e c p"))
        b1s = const.tile([P, EL, 2], f32)
        nc.sync.dma_start(b1s, b1c.rearrange("c p m -> p c m"))
        b2s = const.tile([P, EL, 2], f32)
        nc.sync.dma_start(b2s, b2c.rearrange("c p m -> p c m"))
        b3s = const.tile([P, EL, 4], f32)
        nc.sync.dma_start(b3s, b3c.rearrange("c p m -> p c m"))
        if has_be:
            be1s = const.tile([P, EL, 2], f32)
            nc.sync.dma_start(be1s, be1.rearrange("c p m -> p c m"))
            be2s = const.tile([P, EL, 2], f32)
            nc.sync.dma_start(be2s, be2.rearrange("c p m -> p c m"))
        cds = const.tile([32, 1], f32)
        nc.sync.dma_start(cds, cdot[:, :])
        epsc = const.tile([32, 1], f32)
        nc.vector.memset(epsc, EPS)

        dots_sb = outp.tile([32, B], f32)
        lnssq_sb = outp.tile([32, B], f32)

        for nb in range(NB):
            bsl = ds(nb * CHUNK, CHUNK)
            xts = wpool.tile([P, 4, CHUNK], f32, tag="x")
            nc.sync.dma_start(xts, xt.rearrange("k p b -> p k b")[:, :, bsl])

            svar = pstat.tile([P, CHUNK], f32)
            a1 = [None] * EL
            a2 = [None] * EL

            # ---- phase A: L1 matmuls, bias-copy, square, var1 reduce
            for c in range(EL):
                w1 = wpool.tile([P, 4, HID], f32, tag="w1")
                nc.sync.dma_start(w1, w1t[c].rearrange("k p h -> p k h"))
                p1 = pp.tile([P, 2, CHUNK], f32, tag="p")
                for m in range(2):
                    for k in range(4):
                        nc.tensor.matmul(
                            p1[:, m, :], lhsT=w1[:, k, ds(m * P, P)],
                            rhs=xts[:, k, :], start=(k == 0), stop=(k == 3))
                ta = apool.tile([P, 2, CHUNK], f32, tag="a")
                for m in range(2):
                    nc.vector.tensor_scalar_add(
                        ta[:, m, :], p1[:, m, :], b1s[:, c, m:m + 1])
                tsq = sqpool.tile([P, 2, CHUNK], f32, tag="sq")
                nc.gpsimd.tensor_mul(tsq, ta, ta)
                for m in range(2):
                    nc.tensor.matmul(
                        svar[0:32, :], lhsT=wv1s[:, c, m, :], rhs=tsq[:, m, :],
                        start=(c == 0 and m == 0), stop=(c == EL - 1 and m == 1),
                        tile_position=(0, 0), skip_group_check=True)
                a1[c] = ta

            # ---- phase B: rstd1 = exp(-0.5 ln(var + eps))
            lnv1 = stat.tile([32, CHUNK], f32, tag="ln")
            nc.scalar.activation(lnv1, svar[0:32, :], AF.Ln, bias=epsc)
            rstd1 = stat.tile([32, CHUNK], f32, tag="rstd")
            nc.scalar.activation(rstd1, lnv1, AF.Exp, scale=-0.5)

            # ---- phase C: scale + gelu
            for c in range(EL):
                bc = pt.tile([P, CHUNK], f32, tag="t")
                nc.tensor.matmul(bc, lhsT=ecols[:, c, :], rhs=rstd1,
                                 start=True, stop=True)
                nc.vector.tensor_mul(
                    a1[c], a1[c], bc[:, None, :].to_broadcast([P, 2, CHUNK]))
                if has_be:
                    for m in range(2):
                        nc.vector.tensor_scalar_add(
                            a1[c][:, m, :], a1[c][:, m, :], be1s[:, c, m:m + 1])
                nc.scalar.activation(a1[c], a1[c], AF.Gelu)

            # ---- phase D: L2
            for c in range(EL):
                w2 = wpool.tile([P, 2, HID], f32, tag="w2")
                nc.sync.dma_start(w2, w2t[c].rearrange("k p h -> p k h"))
                p2 = pp.tile([P, 2, CHUNK], f32, tag="p")
                for m in range(2):
                    for k in range(2):
                        nc.tensor.matmul(
                            p2[:, m, :], lhsT=w2[:, k, ds(m * P, P)],
                            rhs=a1[c][:, k, :], start=(k == 0), stop=(k == 1))
                ta = apool.tile([P, 2, CHUNK], f32, tag="a")
                for m in range(2):
                    nc.vector.tensor_scalar_add(
                        ta[:, m, :], p2[:, m, :], b2s[:, c, m:m + 1])
                tsq = sqpool.tile([P, 2, CHUNK], f32, tag="sq")
                nc.gpsimd.tensor_mul(tsq, ta, ta)
                for m in range(2):
                    nc.tensor.matmul(
                        svar[32:64, :], lhsT=wv2s[:, c, m, :], rhs=tsq[:, m, :],
                        start=(c == 0 and m == 0), stop=(c == EL - 1 and m == 1),
                        tile_position=(0, 32), skip_group_check=True)
                a2[c] = ta

            # ---- phase E: rstd2
            lnv2 = stat.tile([32, CHUNK], f32, tag="ln")
            nc.scalar.activation(lnv2, svar[32:64, :], AF.Ln, bias=epsc)
            rstd2 = stat.tile([32, CHUNK], f32, tag="rstd")
            nc.scalar.activation(rstd2, lnv2, AF.Exp, scale=-0.5)

            # ---- phase F: scale + gelu
            for c in range(EL):
                bc = pt.tile([P, CHUNK], f32, tag="t")
                nc.tensor.matmul(bc, lhsT=ecols[:, c, :], rhs=rstd2,
                                 start=True, stop=True)
                nc.vector.tensor_mul(
                    a2[c], a2[c], bc[:, None, :].to_broadcast([P, 2, CHUNK]))
                if has_be:
                    for m in range(2):
                        nc.vector.tensor_scalar_add(
                            a2[c][:, m, :], a2[c][:, m, :], be2s[:, c, m:m + 1])
                nc.scalar.activation(a2[c], a2[c], AF.Gelu)

            # ---- phase G: L3, ssq + dot reduces
            for c in range(EL):
                w3 = wpool.tile([P, 2, OUT], f32, tag="w3")
                nc.sync.dma_start(w3, w3t[c].rearrange("k p h -> p k h"))
                for m in range(4):
                    tt = pt.tile([P, CHUNK], f32, tag="t")
                    for k in range(2):
                        nc.tensor.matmul(
                            tt, lhsT=w3[:, k, ds(m * P, P)], rhs=a2[c][:, k, :],
                            start=(k == 0), stop=(k == 1))
                    tq = sqpool.tile([P, CHUNK], f32, tag="tsq")
                    nc.scalar.activation(tq, tt, AF.Square,
                                         bias=b3s[:, c, m:m + 1])
                    nc.tensor.matmul(
                        svar[96:128, :], lhsT=sws[:, c, m, :], rhs=tq,
                        start=(c == 0 and m == 0), stop=(c == EL - 1 and m == 3),
                        tile_position=(0, 96), skip_group_check=True)
                for k in range(2):
                    nc.tensor.matmul(
                        svar[64:96, :], lhsT=wds[:, c, k, :], rhs=a2[c][:, k, :],
                        start=(c == 0 and k == 0), stop=(c == EL - 1 and k == 1),
                        tile_position=(0, 64), skip_group_check=True)

            # ---- phase H: bank -> output rows
            nc.scalar.activation(lnssq_sb[:, bsl], svar[96:128, :], AF.Ln)
            nc.scalar.copy(dots_sb[:, bsl], svar[64:96, :])

        # ---- final: sim / dist
        rsq = fin.tile([32, B], f32, tag="f")
        nc.scalar.activation(rsq, lnssq_sb, AF.Exp, scale=-0.5)
        nc.vector.tensor_scalar_add(dots_sb, dots_sb, cds)
        simv = fin.tile([32, B], f32, tag="f")
        nc.vector.tensor_mul(simv, dots_sb, rsq)
        import concourse.mybir as _m
        q = fin.tile([32, B], f32, tag="f")
        nc.vector.tensor_scalar(q, simv, -2.0, 2.0,
                                _m.AluOpType.mult, _m.AluOpType.add)
        nc.vector.tensor_scalar_max(q, q, 0.0)
        dst = fin.tile([32, B], f32, tag="f")
        nc.scalar.activation(dst, q, AF.Sqrt)
        nc.sync.dma_start(sim_o[:, :], simv[0:EL, :])
        nc.sync.dma_start(dist_o[:, :], dst[0:EL, :])

    nc.finalize()
    return nc


def _get_prog(has_be):
    global _PROG
    if _PROG is None or _PROG[1] != has_be:
        _PROG = (_build(has_be), has_be)
    return _PROG[0]


# --------------------------------------------------------------------------
# entry point
# --------------------------------------------------------------------------

def kernel(**inputs):
    from concourse.bass_utils import run_bass_kernel_spmd

    per_core, has_be = _prep(inputs)
    nc = _get_prog(has_be)
    res = run_bass_kernel_spmd(nc, per_core, core_ids=list(range(NCORES)))
    sims = np.concatenate([r["sim"] for r in res.results], axis=0)[:C]
    dists = np.concatenate([r["dist"] for r in res.results], axis=0)[:C]
    similarities = np.ascontiguousarray(sims.T)          # [B, C]
    distances = np.ascontiguousarray(dists.T)            # [B, C]
    routed = np.argmin(distances, axis=-1).astype(np.int32)
    return routed, distances, similarities
